# revision 53
# baseline (speedup 1.0000x reference)
"""Trainium2 Bass kernel for a dense self-attention block (B=4, N=S=1024,
C=768, H=12) with an additive attention-weight bias:

    q = heads(x @ Wq.T); k = heads(x @ Wk.T); v = heads(x @ Wv.T)
    attn = softmax(attn_weight + log_softmax(scale * q k^T))
    out  = (attn @ v) @ Wo.T + bo

Math simplifications (exact):
  softmax(w + log_softmax(a)) == softmax(w + a)          (lse shift invariance)
  exp(w + a) == exp(a) * exp(w)  with exp(w) precomputed on HOST.

The second identity removes the in-PE bias-add (an identity-matmul per
attention tile that cost ~25% of all TensorE columns in v1): the device
computes et = exp(qk) on ACT and multiplies elementwise by the streamed
exp(w) tiles on the otherwise-idle Vector engine.

Scheduling principle (HAM): the PE clock-gate only stays at 2.4 GHz while
the PE is ~fully busy, so every S^T window is packed with filler matmul
work (v-projection, qkv m1/m2 chunks, PV bursts) to keep the TensorE FIFO
nonempty; emission order per step is [dense fills] -> qk -> PV -> exp ->
mul -> [late-dependency fills] so a fill waiting on a startup DMA can
never head-of-line-block the qk stream.

Norms: 1/r for pairs 0/1 via the DMA repartition chain (≈11us latency,
hidden: launched 1-2 windows before the result is needed); the final pair
uses the DMA-free exp(-ln r) ACT path + K=1 ones-matmul broadcast.

Sharding: 8 cores = 4 batches x 2 head-groups (6 heads each); host sums
the two half-projections + bias in fp32.
"""

import os
import numpy as np

B, N, C, H = 4, 1024, 768, 12
HG = 2                # head-groups (tensor-parallel factor); cores = B*HG = 8
HPG = H // HG         # heads per group = 6
D = C // H            # 64
GJ = HPG * D          # 384
P = 128
SC = N // P           # 8 s-chunks of 128
MQ = GJ // P          # 3 row chunks of qT/kT
KC = C // P           # 6 contraction chunks over C
NB2 = N // 512        # 2 column chunks of 512
NCORES = B * HG
SCALE = D ** -0.5

NWARM = int(os.environ.get("K_NWARM", "76"))
EW_BUFS = 9
ET_BUFS = 18
ERAW_BUFS = 6


def build_program():
    """Build and compile the per-core Bass program. Returns the Bacc object."""
    import concourse.bass as bass
    import concourse.mybir as mybir
    import concourse.tile as tile
    from concourse import bacc

    nc = bacc.Bacc(
        "TRN2",
        target_bir_lowering=False,
        debug=False,
        num_devices=NCORES,
    )
    f32 = mybir.dt.float32
    f16 = mybir.dt.float16
    EXP = mybir.ActivationFunctionType.Exp
    LOG = mybir.ActivationFunctionType.Ln

    xT_d = nc.dram_tensor("xT", [C, N], f16, kind="ExternalInput").ap()
    wqk_d = nc.dram_tensor("wqk", [C, 2 * GJ], f16, kind="ExternalInput").ap()
    wvT_d = nc.dram_tensor("wvT", [C, GJ], f16, kind="ExternalInput").ap()
    woT_d = nc.dram_tensor("woT", [GJ, C], f16, kind="ExternalInput").ap()
    ew_d = nc.dram_tensor("ew", [HPG, N, N], f16, kind="ExternalInput").ap()
    out_d = nc.dram_tensor("out", [N, C], f16, kind="ExternalOutput").ap()

    def mm(out, lhsT, rhs, start, stop):
        nc.tensor.matmul(out, lhsT, rhs, start=start, stop=stop)

    with tile.TileContext(nc) as tc:
        with (
            tc.tile_pool(name="const", bufs=1) as const_pool,
            tc.tile_pool(name="ewtile", bufs=EW_BUFS) as ew_pool,
            tc.tile_pool(name="eraw", bufs=ERAW_BUFS) as eraw_pool,
            tc.tile_pool(name="etile", bufs=ET_BUFS) as e_pool,
            tc.tile_pool(name="rtile", bufs=4) as r_pool,
            tc.tile_pool(name="rbtile", bufs=2) as rb_pool,
            tc.tile_pool(name="vcptile", bufs=3) as vcp_pool,
            tc.tile_pool(name="outtile", bufs=2) as out_pool,
            tc.tile_pool(name="ps_s", bufs=2, space="PSUM") as psum_s,
            tc.tile_pool(name="ps_o", bufs=4, space="PSUM") as psum_o,
            tc.tile_pool(name="dram", bufs=4, space="DRAM") as dram_pool,
        ):
            # ---- constants / fill -----------------------------------------
            # x / wqk in 3-chunk groups (768/576 KB transfers: big enough
            # for ~75% DMA efficiency), interleaved g0-first on two queues
            # so the first contraction chunks land earliest and the QKV-m0
            # matmuls stream right behind the fill.
            warm_sb = const_pool.tile([P, P], f16)
            nc.gpsimd.memset(warm_sb, 0.0)
            ones_sb = const_pool.tile([P, P], f16)
            nc.gpsimd.memset(ones_sb, 1.0)

            xg = [const_pool.tile([P, 3, N], f16, name=f"xg{g}")
                  for g in range(2)]
            wqkg = [const_pool.tile([P, 3, 2 * GJ], f16, name=f"wqkg{g}")
                    for g in range(2)]
            xT_r = xT_d.rearrange("(o p) n -> p o n", p=P)
            wqk_r = wqk_d.rearrange("(o p) j -> p o j", p=P)
            for g in range(2):
                nc.sync.dma_start(xg[g], xT_r[:, 3 * g:3 * g + 3])
                nc.scalar.dma_start(wqkg[g], wqk_r[:, 3 * g:3 * g + 3])
            x_sbs = [xg[k // 3][:, k % 3, :] for k in range(KC)]
            wqk_sbs = [wqkg[k // 3][:, k % 3, :] for k in range(KC)]

            # wv / woT on the sync queue, gated behind the last x group (a
            # tiny DMA with a real data dependency — the list scheduler
            # cannot hoist the loads ahead of the fill, and same-queue
            # FIFO then orders the transfers after the gate fires).
            wvg = [const_pool.tile([P, 3, GJ], f16, name=f"wvg{g}")
                   for g in range(2)]
            woT_sb = const_pool.tile([P, MQ, C], f16)
            # wv gated on the FIRST x group: it lands mid-fill so the
            # v-projection can start right after QKV-m0 (covering the
            # PE-idle window while the m0 casts run).
            nc.sync.dma_start(wvg[0][0:1, 0:1, 0:64], xg[0][0:1, 0, 0:64])
            wv_r = wvT_d.rearrange("(o p) j -> p o j", p=P)
            for g in range(2):
                nc.sync.dma_start(wvg[g], wv_r[:, 3 * g:3 * g + 3])
            nc.sync.dma_start(woT_sb,
                              woT_d.rearrange("(o p) c -> p o c", p=P))
            wv_sbs = [wvg[k // 3][:, k % 3, :] for k in range(KC)]

            # gate the ew stream the same way (dummy pool tiles; real ew
            # DMAs rotate into these slots, semaphore-ordered after fill)
            for i in range(EW_BUFS):
                g = ew_pool.tile([P, 2, N], f16, tag="ew", name=f"ewgate{i}")
                nc.gpsimd.dma_start(g[0:1, 0:1, 0:64], xg[1][0:1, 0, 0:64])
            # ew view: [sc, p, h, n]
            ew_r = ew_d.rearrange("h (c p) n -> c p h n", p=P)

            # ---- PE warmup ------------------------------------------------
            # Zero-matmul stream covering preamble -> first-group landing,
            # so the HAM clock-gate un-throttles into the QKV-m0 stream.
            warm_ps = psum_s.tile([P, N], f32, tag="ps_s")
            for i in range(NWARM):
                mm(warm_ps[:, 0:P], warm_sb, warm_sb,
                   start=(i == 0), stop=(i == NWARM - 1))

            qT_sbs = [const_pool.tile([P, N], f16, name=f"qT{j}")
                      for j in range(MQ)]
            kT_sbs = [const_pool.tile([P, N], f16, name=f"kT{j}")
                      for j in range(MQ)]
            oT_sbs = [const_pool.tile([P, N], f16, name=f"oT{j}")
                      for j in range(MQ)]
            # [v_h | 1 | 0...] (even heads use cols 0:65) /
            # [0... | 1 | 0 | v_h] (odd heads use cols 0:128, one at col 32)
            v_aug = const_pool.tile([P, SC, HPG, P], f16)
            for h in range(HPG):
                if h % 2 == 0:
                    nc.scalar.memzero(v_aug[:, :, h, 64:66])
                    nc.scalar.add(v_aug[:, :, h, 64:65],
                                  v_aug[:, :, h, 64:65], 1.0)
                else:
                    nc.scalar.memzero(v_aug[:, :, h, 0:64])
                    nc.scalar.add(v_aug[:, :, h, 32:33],
                                  v_aug[:, :, h, 32:33], 1.0)

            # ---- QKV projections ------------------------------------------
            # m0 streams behind the fill; casts split in halves so pair 0's
            # first steps unblock one ACT-copy earlier.
            def qkv_m0():
                ps_q = psum_s.tile([P, N], f32, tag="ps_s", name="qkv_q0")
                ps_k = psum_s.tile([P, N], f32, tag="ps_s", name="qkv_k0")
                for kc in range(KC):
                    # k first: pair 0's first qk LDWEIGHTS needs kT
                    for ps, j0 in ((ps_k, GJ), (ps_q, 0)):
                        for nb in range(NB2):
                            ncol = slice(nb * 512, (nb + 1) * 512)
                            mm(ps[:, ncol],
                               wqk_sbs[kc][:, j0:j0 + P],
                               x_sbs[kc][:, ncol],
                               start=(kc == 0), stop=(kc == KC - 1))
                # casts split in halves, k first (pair 0's first qk needs
                # kT's low half before qT's high half)
                for half in range(NB2):
                    ncol = slice(half * 512, (half + 1) * 512)
                    nc.scalar.copy(kT_sbs[0][:, ncol], ps_k[:, ncol])
                    nc.scalar.copy(qT_sbs[0][:, ncol], ps_q[:, ncol])

            qkv_m0()

            def qkv_m1(m, which):
                """Emit one of q/k for row-chunk m (1 psum slot borrow)."""
                j0 = m * P if which == "q" else GJ + m * P
                dst = qT_sbs[m] if which == "q" else kT_sbs[m]
                ps = psum_s.tile([P, N], f32, tag="ps_s",
                                 name=f"qkv_{which}{m}")
                for nb in range(NB2):
                    ncol = slice(nb * 512, (nb + 1) * 512)
                    for kc in range(KC):
                        mm(ps[:, ncol], wqk_sbs[kc][:, j0:j0 + P],
                           x_sbs[kc][:, ncol],
                           start=(kc == 0), stop=(kc == KC - 1))
                # split the two casts of each m-chunk across engines (a
                # full-tile DVE cast costs 1.7us, an ACT one 1.15us; both
                # engines are near their window budgets)
                if which == "q":
                    nc.scalar.copy(dst[:], ps)
                else:
                    nc.vector.tensor_copy(dst[:], ps)

            def emit_v(sc):
                # v-projection for one s-chunk on a ps_o slot; ScalarE
                # scatters the result into v_aug (even cols 0:64 / odd
                # cols 64:128 per head).
                ps = psum_o.tile([P, 512], f32, tag="ps_o", name=f"ps_v{sc}")
                for kc in range(KC):
                    mm(ps[:, :GJ],
                       x_sbs[kc][:, sc * P:(sc + 1) * P],
                       wv_sbs[kc][:, :],
                       start=(kc == 0), stop=(kc == KC - 1))
                vsrc = ps[:, :GJ].rearrange("p (h d) -> p h d", d=D)
                nc.vector.tensor_copy(v_aug[:, sc, 0:HPG:2, 0:64],
                                      vsrc[:, 0:HPG:2, :])
                nc.vector.tensor_copy(v_aug[:, sc, 1:HPG:2, 64:128],
                                      vsrc[:, 1:HPG:2, :])

            # ---- attention pair loop --------------------------------------
            def st_pair(hp, pv_sched=None, pv_emit=None, hooks_pre=None,
                        hooks=None, ets_out=None, prefetch_ew=False,
                        gp_mul_scs=()):
                """One even/odd head pair's qk -> exp -> (x ew) stream.
                hooks_pre[sc]: dense PE fills emitted BEFORE the qk mms
                (must not depend on late startup DMAs).  hooks[sc]: work
                emitted after the step's mul (may have late deps).
                pv_sched[sc] PV groups of pv_emit interleave after qk."""
                j = hp // 2
                qe, ke = qT_sbs[j][0:64, :], kT_sbs[j][0:64, :]
                qo, ko = qT_sbs[j][64:128, :], kT_sbs[j][64:128, :]
                ets_e, ets_o = ([], []) if ets_out is None else ets_out
                pv_next = 0
                ew_ts = []
                if prefetch_ew:
                    # split across two queues so the 4MB burst drains two
                    # rings in parallel; sync (whose FIFO also carries the
                    # pair-1 norm-chain hops) gets the late-needed tiles
                    for sc in range(SC):
                        ew_t = ew_pool.tile([P, 2, N], f16, tag="ew")
                        dq = nc.gpsimd if sc < 4 else nc.sync
                        dq.dma_start(ew_t, ew_r[sc][:, hp:hp + 2, :])
                        ew_ts.append(ew_t)
                for sc in range(SC):
                    scol = slice(sc * P, (sc + 1) * P)
                    if prefetch_ew:
                        ew_t = ew_ts[sc]
                    else:
                        ew_t = ew_pool.tile([P, 2, N], f16, tag="ew")
                        nc.gpsimd.dma_start(ew_t, ew_r[sc][:, hp:hp + 2, :])
                    if hooks_pre is not None and sc in hooks_pre:
                        for fn in hooks_pre[sc]:
                            fn()
                    ps_e = psum_s.tile([P, N], f32, tag="ps_s")
                    ps_o = psum_s.tile([P, N], f32, tag="ps_s")
                    # each 512-col psum region is written by exactly one
                    # matmul -> start+stop per region
                    for nb in range(NB2):
                        ncol = slice(nb * 512, (nb + 1) * 512)
                        mm(ps_e[:, ncol], ke[:, scol], qe[:, ncol],
                           start=True, stop=True)
                    for nb in range(NB2):
                        ncol = slice(nb * 512, (nb + 1) * 512)
                        mm(ps_o[:, ncol], ko[:, scol], qo[:, ncol],
                           start=True, stop=True)
                    if pv_emit is not None and pv_sched is not None:
                        for _ in range(pv_sched[sc]):
                            pv_emit(pv_next)
                            pv_next += 1
                    # last step: odd head first — its exp->mul->PV chain
                    # gates the tail's 1/r path
                    order = ((1, ps_o), (0, ps_e)) if sc == SC - 1 \
                        else ((0, ps_e), (1, ps_o))
                    ets = {}
                    for par, ps in order:
                        er = eraw_pool.tile([P, N], f16, tag="eraw")
                        nc.scalar.activation(er, ps, EXP)
                        eng = (nc.gpsimd if (sc in gp_mul_scs and par == 1)
                               else nc.vector)
                        et = e_pool.tile([P, N], f16, tag="et")
                        eng.tensor_mul(et, er, ew_t[:, par, :])
                        ets[par] = et
                    et_e, et_o = ets[0], ets[1]
                    ets_e.append(et_e)
                    ets_o.append(et_o)
                    if hooks is not None and sc in hooks:
                        for fn in hooks[sc]:
                            fn()
                return ets_e, ets_o, pv_next

            def make_pv(hp, ets_pair):
                halves = {}
                for h, _ in ets_pair:
                    for nb in range(NB2):
                        halves[(h, nb)] = psum_o.tile([P, 512], f32,
                                                      tag="ps_o",
                                                      name=f"pso_h{h}_n{nb}")

                def emit(sc):
                    for h, etiles in ets_pair:
                        even = (h % 2 == 0)
                        lh = (v_aug[:, sc, h, 0:65] if even
                              else v_aug[:, sc, h, 0:P])
                        for nb in range(NB2):
                            ncol = slice(nb * 512, (nb + 1) * 512)
                            pso = halves[(h, nb)]
                            po = (pso[0:65, :] if even else pso[:, :])
                            mm(po, lh, etiles[sc][:, ncol],
                               start=(sc == 0), stop=(sc == SC - 1))

                def emit_h(which, sc):
                    h, etiles = ets_pair[which]
                    even = (h % 2 == 0)
                    lh = (v_aug[:, sc, h, 0:65] if even
                          else v_aug[:, sc, h, 0:P])
                    for nb in range(NB2):
                        ncol = slice(nb * 512, (nb + 1) * 512)
                        pso = halves[(h, nb)]
                        po = (pso[0:65, :] if even else pso[:, :])
                        mm(po, lh, etiles[sc][:, ncol],
                           start=(sc == 0), stop=(sc == SC - 1))

                h_e, h_o = ets_pair[0][0], ets_pair[1][0]
                return emit, ([halves[(h_e, 0)], halves[(h_e, 1)]],
                              [halves[(h_o, 0)], halves[(h_o, 1)]]), emit_h

            # Norm chain (pairs 0/1), split into 3 stages so each DVE op's
            # inputs are resolved before it reaches the head of the
            # strict-FIFO DVE queue (an unresolved reciprocal would block
            # the exp-tile multiply stream for ~2us).  End-to-end latency
            # is ~11us (4 HBM round trips) — launched 1.5 windows before
            # the tail needs oT.
            def norm_a(h, halves, dq):
                # ONE copy per psum half evacuates the PV rows AND the r
                # row together (even head: psum rows 0:65 hold v|r; odd:
                # rows 32:128 hold r|..|v) — r-only copies would be
                # single-partition and ~0.7us each on DVE, which congests
                # the pair boundary.
                off = (h % 2) * 64
                rrow = 64 if h % 2 == 0 else 32
                vcp = vcp_pool.tile([P, N], f32, tag="vcp")
                for nb, pso in enumerate(halves):
                    ncol = slice(nb * 512, (nb + 1) * 512)
                    nc.vector.tensor_copy(vcp[:, ncol], pso)
                rd1 = dram_pool.tile([1, N], f32, tag="rd1")
                dq.dma_start(rd1, vcp[rrow:rrow + 1, :])
                rsq = r_pool.tile([P, N // P], f32, tag="rsq")
                dq.dma_start(
                    rsq, rd1.rearrange("one (p o) -> (one p) o", p=P))
                return rsq, vcp

            def norm_b(h, st, dq):
                rsq, vcp = st
                off = (h % 2) * 64
                nc.vector.reciprocal(rsq, rsq)
                rd2 = dram_pool.tile([1, N], f32, tag="rd2")
                dq.dma_start(
                    rd2.rearrange("one (p o) -> (one p) o", p=P), rsq)
                rb = rb_pool.tile([P, N], f32, tag="rb")
                dq.dma_start(rb[off:off + 64, :],
                             rd2[0:1, :].partition_broadcast(64))
                return rb, vcp

            def norm_c(h, st, eng=None):
                rb, vcp = st
                eng = eng or nc.vector
                off = (h % 2) * 64
                for nb in range(NB2):
                    ncol = slice(nb * 512, (nb + 1) * 512)
                    eng.tensor_mul(
                        oT_sbs[h // 2][off:off + 64, ncol],
                        vcp[off:off + 64, ncol],
                        rb[off:off + 64, ncol])

            # ---- the three pair windows -----------------------------------
            # v(0)/v(1) pre-emitted: they fill the PE-idle window between
            # the last m0 matmul and pair-0's first qk (blocked on the m0
            # casts); wv was gated on the first x group so it has landed.
            emit_v(0)
            emit_v(1)

            # pair 0 (heads 0/1): every step carries >=4k filler columns
            # (HAM re-warm needs fully-busy 3.4us windows): v chunks at
            # steps 0-3 and 6-7, the m1 groups at 4/5.  v(6)/v(7) are only
            # consumed by the PV0 burst late in pair 1, so they can land
            # last.
            hooks0_pre = {4: [lambda: qkv_m1(1, "q")],
                          5: [lambda: qkv_m1(1, "k")]}
            hooks0 = {0: [lambda: emit_v(2)], 1: [lambda: emit_v(3)],
                      2: [lambda: emit_v(4)], 3: [lambda: emit_v(5)],
                      6: [lambda: emit_v(6)], 7: [lambda: emit_v(7)]}
            ets0_e, ets0_o, _ = st_pair(0, hooks_pre=hooks0_pre,
                                        hooks=hooks0)

            # pair 1 (heads 2/3): PV(pair 0) spread (closes step 5), m2
            # fills at 5/6; pair-0 evacuations right at close (steps 5/6)
            # so the ps_o slots are free before pair-2's PV allocates;
            # the ~11us chain latency resolves mid-pair-2.
            st = {}
            pv0, (h0_e, h0_o), _ = make_pv(0, [(0, ets0_e), (1, ets0_o)])
            hooks1_pre = {
                5: [lambda: qkv_m1(2, "q")],
                # a(1) as a PRE-hook: its DVE copies run before step 6's
                # et-muls so the freed ps_o slots (and the mul stream) are
                # not held behind each other at the pair boundary.
                6: [lambda: st.__setitem__(1, norm_a(1, h0_o, nc.sync)),
                    lambda: qkv_m1(2, "k")],
            }
            hooks1 = {
                5: [lambda: st.__setitem__(0, norm_a(0, h0_e, nc.sync))],
                6: [lambda: st.__setitem__(0, norm_b(0, st[0], nc.sync))],
                7: [lambda: st.__setitem__(1, norm_b(1, st[1], nc.sync))],
            }
            ets1_e, ets1_o, _ = st_pair(
                2, pv_sched=[2, 2, 1, 1, 1, 1, 0, 0], pv_emit=pv0,
                hooks_pre=hooks1_pre, hooks=hooks1)

            # pair 2 (heads 4/5): all 8 ew tiles prefetched at step 0 on
            # two queues.  PV(1) burst closes at step 3; pair-1 norms
            # staged 4-7 (sync + gpsimd in parallel, landing ~tail+1);
            # pair-2's own PV self-lags from step 5 into the freed ps_o
            # slots.  Pair-0's final muls at steps 2/3 (their rb lands
            # around step 2 — emitted later so the strict-FIFO DVE queue
            # never parks on them).
            pv1, (h1_e, h1_o), pv1_h = make_pv(1, [(2, ets1_e), (3, ets1_o)])
            pv2_emit_holder = []

            def start_pv2():
                emit2, halves, _ = make_pv(2, [(4, ets2_e), (5, ets2_o)])
                pv2_emit_holder.append((emit2, halves))

            ets2_e, ets2_o = [], []

            def pv2(i):
                pv2_emit_holder[0][0](i)

            # DVE is the scarce engine in this window (et-muls + psum
            # evacuations + reciprocals ≈ its full budget): all six final
            # oT muls ride the idle GpSimd instead, and the pair-1 norm
            # chains ride gpsimd/sync where nothing queues behind them.
            # PV2 closes IN-LOOP at step 7 so the tail's 1/r chain starts
            # immediately; steps 5-7 are packed with its burst.
            hooks2_pre = {
                4: [lambda: st.__setitem__(2, norm_a(2, h1_e, nc.gpsimd))],
            }
            hooks2 = {
                2: [lambda: norm_c(0, st[0], nc.gpsimd)],
                3: [lambda: norm_c(1, st[1], nc.gpsimd)],
                4: [lambda: st.__setitem__(3, norm_a(3, h1_o, nc.sync))],
                5: [lambda: st.__setitem__(2, norm_b(2, st[2], nc.gpsimd)),
                    lambda: st.__setitem__(3, norm_b(3, st[3], nc.sync)),
                    start_pv2, lambda: pv2(0), lambda: pv2(1)],
                6: [lambda: norm_c(2, st[2], nc.gpsimd),
                    lambda: pv2(2), lambda: pv2(3), lambda: pv2(4)],
                7: [lambda: norm_c(3, st[3], nc.gpsimd),
                    lambda: pv2(5), lambda: pv2(6), lambda: pv2(7)],
            }
            for sc in range(4):
                hooks2.setdefault(0, []).append(
                    lambda sc=sc: pv1_h(0, sc))
                hooks2.setdefault(1, []).append(
                    lambda sc=sc: pv1_h(1, sc))
                hooks2.setdefault(2, []).append(
                    lambda sc=sc: pv1_h(0, sc + 4))
                hooks2.setdefault(3, []).append(
                    lambda sc=sc: pv1_h(1, sc + 4))
            st_pair(4, hooks_pre=hooks2_pre, hooks=hooks2,
                    ets_out=(ets2_e, ets2_o), prefetch_ew=True)
            _, (h2_e, h2_o) = pv2_emit_holder[0]

            # ---- tail -----------------------------------------------------

            def oproj_mms(nb, ps0, ps1, j3s, start, stop):
                for cb, ps in ((0, ps0), (1, ps1)):
                    cw = 512 if cb == 0 else C - 512
                    for j3 in j3s:
                        mm(ps[:, 0:cw],
                           oT_sbs[j3][:, nb * P:(nb + 1) * P],
                           woT_sb[:, j3, cb * 512:cb * 512 + cw],
                           start=(start and j3 == j3s[0]),
                           stop=(stop and j3 == j3s[-1]))

            # oproj j3=0,1 pre-run for nb 0/1 on the freed ps_s slots: PE
            # work that covers the final norm's ACT latency.
            pre = {}
            for nb in range(2):
                psw = psum_s.tile([P, N], f32, tag="ps_s", name=f"pow_{nb}")
                pre[nb] = (psw[:, 0:512], psw[:, 512:1024])
                oproj_mms(nb, pre[nb][0], pre[nb][1], [0, 1], True, False)
            # nb2 pre-runs on TWO ps_o slots (freed by the evacuations
            # below); the 1/r broadcasts use the other two, per-head
            # sequentially — no slot set is ever held against oT[2].
            po2 = (psum_o.tile([P, 512], f32, tag="ps_o", name="po0_2"),
                   psum_o.tile([P, 512], f32, tag="ps_o", name="po1_2"))
            pre[2] = po2
            oproj_mms(2, po2[0], po2[1], [0, 1], True, False)

            # Final pair's norm, DMA-free: 1/r = exp(-ln r) on ACT.  Both
            # heads' r rows (psum partitions 64 / 32) are copied into ONE
            # tile so a SINGLE Ln and a SINGLE Exp over partition rows
            # 32:65 cover both heads — one table-set switch each instead
            # of the 4 the scheduler otherwise interleaves (rows 33:63
            # process garbage; their outputs are never read).  A K=1
            # ones-matmul then broadcasts each head's 1/r row across its
            # 64 output partitions via PSUM.
            # ALL r-row copies on ACT: a DVE r-copy at the strict-FIFO head
            # would block the final et-multiplies (which gate PV2's close);
            # ACT is otherwise idle here and the Ln follows on the same
            # queue.  Odd head (row 32) first — its PV closed first.
            r_t = r_pool.tile([P, N], f32, tag="r", name="rt_tail")
            for h, halves in ((5, h2_o), (4, h2_e)):
                rrow = 64 if h % 2 == 0 else 32
                for nb, pso in enumerate(halves):
                    ncol = slice(nb * 512, (nb + 1) * 512)
                    nc.scalar.copy(r_t[rrow:rrow + 1, ncol],
                                   pso[rrow:rrow + 1, :])
            vcps = {}
            for h, halves in ((4, h2_e), (5, h2_o)):
                off = (h % 2) * 64
                vcp = vcp_pool.tile([P, N], f32, tag="vcp", name=f"vcpt{h}")
                for nb, pso in enumerate(halves):
                    ncol = slice(nb * 512, (nb + 1) * 512)
                    nc.vector.tensor_copy(vcp[off:off + 64, ncol],
                                          pso[off:off + 64, :])
                vcps[h] = vcp
            # full-height Ln/Exp: ACT time is free-dim-paced, so covering
            # all 128 partitions costs the same as 2 rows and keeps the
            # AP legal; rows other than 32/64 process garbage, unread.
            rln_t = r_pool.tile([P, N], f32, tag="rsq2", name="rln_tail")
            nc.scalar.activation(rln_t, r_t, LOG)
            rinv_t = r_pool.tile([P, N], f16, tag="rfl", name="rinv_tail")
            nc.scalar.activation(rinv_t, rln_t, EXP, scale=-1.0)
            # 1/r broadcast via K=1 ones-matmuls into the ps_o slots the
            # evacuations just freed, then the oT[2] muls on DVE.
            for h in (4, 5):
                off = (h % 2) * 64
                rrow = 64 if h % 2 == 0 else 32
                for nb in range(NB2):
                    ncol = slice(nb * 512, (nb + 1) * 512)
                    rbp = psum_o.tile([P, 512], f32, tag="ps_o",
                                      name=f"rbp{h}_{nb}")
                    mm(rbp[off:off + 64, :], ones_sb[rrow:rrow + 1, 0:64],
                       rinv_t[rrow:rrow + 1, ncol], start=True, stop=True)
                    nc.vector.tensor_mul(
                        oT_sbs[h // 2][off:off + 64, ncol],
                        vcps[h][off:off + 64, ncol],
                        rbp[off:off + 64, :])

            def oproj_evac(nb, ps0, ps1):
                # evacuations pace the tail if they all ride one engine:
                # split halves across DVE and ACT (the DVE psum->fp16 cast
                # is contiguous here, which the hardware handles; only
                # strided casts mis-stride).
                ob = out_pool.tile([P, C], f16, tag="ob")
                nc.vector.tensor_copy(ob[:, 0:512], ps0)
                nc.scalar.copy(ob[:, 512:C], ps1[:, 0:C - 512])
                nc.sync.dma_start(
                    out_d.rearrange("(o p) c -> o p c", p=P)[nb], ob)

            for nb in range(SC):
                if nb in pre:
                    ps0, ps1 = pre[nb]
                    oproj_mms(nb, ps0, ps1, [2], False, True)
                else:
                    ps0 = psum_o.tile([P, 512], f32, tag="ps_o",
                                      name=f"po0_{nb}")
                    ps1 = psum_o.tile([P, 512], f32, tag="ps_o",
                                      name=f"po1_{nb}")
                    oproj_mms(nb, ps0, ps1, [0, 1, 2], True, True)
                oproj_evac(nb, ps0, ps1)

    nc.compile()
    return nc


_PROG = None


def _get_prog():
    global _PROG
    if _PROG is None:
        _PROG = build_program()
    return _PROG


def make_in_maps(query, attn_weight, Wq, Wk, Wv, Wo):
    query = np.asarray(query, dtype=np.float32)
    attn_weight = np.asarray(attn_weight, dtype=np.float32)
    Wq = np.asarray(Wq, dtype=np.float32)
    Wk = np.asarray(Wk, dtype=np.float32)
    Wv = np.asarray(Wv, dtype=np.float32)
    Wo = np.asarray(Wo, dtype=np.float32)

    in_maps = []
    for b in range(B):
        xT = np.ascontiguousarray(query[b].T).astype(np.float16)
        for g in range(HG):
            rows = slice(g * GJ, (g + 1) * GJ)
            wqk = np.ascontiguousarray(np.concatenate(
                [(SCALE * Wq[rows, :]).T, Wk[rows, :].T],
                axis=1)).astype(np.float16)
            wvT = np.ascontiguousarray(Wv[rows, :].T).astype(np.float16)
            woT = np.ascontiguousarray(Wo[:, rows].T).astype(np.float16)
            ew = np.exp(np.ascontiguousarray(
                attn_weight[b, g * HPG:(g + 1) * HPG].transpose(0, 2, 1))
            ).astype(np.float16)
            in_maps.append({
                "xT": xT, "wqk": wqk, "wvT": wvT, "woT": woT, "ew": ew,
            })
    return in_maps


def run(inputs, trace=False, **spmd_kwargs):
    """Execute on 8 cores; returns (full_output, BassKernelResults)."""
    from concourse import bass_utils

    nc = _get_prog()
    in_maps = make_in_maps(inputs["query"], inputs["attn_weight"],
                           inputs["Wq"], inputs["Wk"], inputs["Wv"],
                           inputs["Wo"])
    res = bass_utils.run_bass_kernel_spmd(
        nc, in_maps, core_ids=list(range(NCORES)), trace=trace, **spmd_kwargs)
    bo = np.asarray(inputs["bo"], dtype=np.float32)
    full = np.empty((B, N, C), dtype=np.float32)
    for b in range(B):
        full[b] = (res.results[2 * b]["out"].astype(np.float32)
                   + res.results[2 * b + 1]["out"].astype(np.float32) + bo)
    return full, res


def kernel(**inputs):
    full, _ = run(inputs, trace=False)
    return full


# revision 54
# speedup vs baseline: 1.0484x; 1.0484x over previous
"""Trainium2 Bass kernel for a dense self-attention block (B=4, N=S=1024,
C=768, H=12) with an additive attention-weight bias:

    q = heads(x @ Wq.T); k = heads(x @ Wk.T); v = heads(x @ Wv.T)
    attn = softmax(attn_weight + log_softmax(scale * q k^T))
    out  = (attn @ v) @ Wo.T + bo

Math simplifications (exact):
  softmax(w + log_softmax(a)) == softmax(w + a)          (lse shift invariance)
  exp(w + a) == exp(a) * exp(w)  with exp(w) precomputed on HOST.

The second identity removes the in-PE bias-add (an identity-matmul per
attention tile that cost ~25% of all TensorE columns in v1): the device
computes et = exp(qk) on ACT and multiplies elementwise by the streamed
exp(w) tiles on the otherwise-idle Vector engine.

Scheduling principle (HAM): the PE clock-gate only stays at 2.4 GHz while
the PE is ~fully busy, so every S^T window is packed with filler matmul
work (v-projection, qkv m1/m2 chunks, PV bursts) to keep the TensorE FIFO
nonempty; emission order per step is [dense fills] -> qk -> PV -> exp ->
mul -> [late-dependency fills] so a fill waiting on a startup DMA can
never head-of-line-block the qk stream.

Norms: 1/r for pairs 0/1 via the DMA repartition chain (≈11us latency,
hidden: launched 1-2 windows before the result is needed); the final pair
uses the DMA-free exp(-ln r) ACT path + K=1 ones-matmul broadcast.

Sharding: 8 cores = 4 batches x 2 head-groups (6 heads each); host sums
the two half-projections + bias in fp32.
"""

import os
import numpy as np

B, N, C, H = 4, 1024, 768, 12
HG = 2                # head-groups (tensor-parallel factor); cores = B*HG = 8
HPG = H // HG         # heads per group = 6
D = C // H            # 64
GJ = HPG * D          # 384
P = 128
SC = N // P           # 8 s-chunks of 128
MQ = GJ // P          # 3 row chunks of qT/kT
KC = C // P           # 6 contraction chunks over C
NB2 = N // 512        # 2 column chunks of 512
NCORES = B * HG
SCALE = D ** -0.5

NWARM = int(os.environ.get("K_NWARM", "76"))
EW_BUFS = 9
ET_BUFS = 18
ERAW_BUFS = 6


def build_program():
    """Build and compile the per-core Bass program. Returns the Bacc object."""
    import concourse.bass as bass
    import concourse.mybir as mybir
    import concourse.tile as tile
    from concourse import bacc

    nc = bacc.Bacc(
        "TRN2",
        target_bir_lowering=False,
        debug=False,
        num_devices=NCORES,
    )
    f32 = mybir.dt.float32
    f16 = mybir.dt.float16
    EXP = mybir.ActivationFunctionType.Exp
    LOG = mybir.ActivationFunctionType.Ln

    xT_d = nc.dram_tensor("xT", [C, N], f16, kind="ExternalInput").ap()
    wqk_d = nc.dram_tensor("wqk", [C, 2 * GJ], f16, kind="ExternalInput").ap()
    wvT_d = nc.dram_tensor("wvT", [C, GJ], f16, kind="ExternalInput").ap()
    woT_d = nc.dram_tensor("woT", [GJ, C], f16, kind="ExternalInput").ap()
    ew_d = nc.dram_tensor("ew", [HPG, N, N], f16, kind="ExternalInput").ap()
    out_d = nc.dram_tensor("out", [N, C], f16, kind="ExternalOutput").ap()

    def mm(out, lhsT, rhs, start, stop):
        nc.tensor.matmul(out, lhsT, rhs, start=start, stop=stop)

    with tile.TileContext(nc) as tc:
        with (
            tc.tile_pool(name="const", bufs=1) as const_pool,
            tc.tile_pool(name="ewtile", bufs=EW_BUFS) as ew_pool,
            tc.tile_pool(name="eraw", bufs=ERAW_BUFS) as eraw_pool,
            tc.tile_pool(name="etile", bufs=ET_BUFS) as e_pool,
            tc.tile_pool(name="rtile", bufs=4) as r_pool,
            tc.tile_pool(name="rbtile", bufs=2) as rb_pool,
            tc.tile_pool(name="vcptile", bufs=3) as vcp_pool,
            tc.tile_pool(name="outtile", bufs=2) as out_pool,
            tc.tile_pool(name="ps_s", bufs=2, space="PSUM") as psum_s,
            tc.tile_pool(name="ps_o", bufs=4, space="PSUM") as psum_o,
            tc.tile_pool(name="dram", bufs=4, space="DRAM") as dram_pool,
        ):
            # ---- constants / fill -----------------------------------------
            # x / wqk in 3-chunk groups (768/576 KB transfers: big enough
            # for ~75% DMA efficiency), interleaved g0-first on two queues
            # so the first contraction chunks land earliest and the QKV-m0
            # matmuls stream right behind the fill.
            warm_sb = const_pool.tile([P, P], f16)
            nc.gpsimd.memset(warm_sb, 0.0)
            ones_sb = const_pool.tile([P, P], f16)
            nc.gpsimd.memset(ones_sb, 1.0)

            xg = [const_pool.tile([P, 3, N], f16, name=f"xg{g}")
                  for g in range(2)]
            wqkg = [const_pool.tile([P, 3, 2 * GJ], f16, name=f"wqkg{g}")
                    for g in range(2)]
            xT_r = xT_d.rearrange("(o p) n -> p o n", p=P)
            wqk_r = wqk_d.rearrange("(o p) j -> p o j", p=P)
            for g in range(2):
                nc.sync.dma_start(xg[g], xT_r[:, 3 * g:3 * g + 3])
                nc.scalar.dma_start(wqkg[g], wqk_r[:, 3 * g:3 * g + 3])
            x_sbs = [xg[k // 3][:, k % 3, :] for k in range(KC)]
            wqk_sbs = [wqkg[k // 3][:, k % 3, :] for k in range(KC)]

            # wv / woT on the sync queue, gated behind the last x group (a
            # tiny DMA with a real data dependency — the list scheduler
            # cannot hoist the loads ahead of the fill, and same-queue
            # FIFO then orders the transfers after the gate fires).
            wvg = [const_pool.tile([P, 3, GJ], f16, name=f"wvg{g}")
                   for g in range(2)]
            woT_sb = const_pool.tile([P, MQ, C], f16)
            # wv gated on the FIRST x group: it lands mid-fill so the
            # v-projection can start right after QKV-m0 (covering the
            # PE-idle window while the m0 casts run).
            nc.sync.dma_start(wvg[0][0:1, 0:1, 0:64], xg[0][0:1, 0, 0:64])
            wv_r = wvT_d.rearrange("(o p) j -> p o j", p=P)
            for g in range(2):
                nc.sync.dma_start(wvg[g], wv_r[:, 3 * g:3 * g + 3])
            nc.sync.dma_start(woT_sb,
                              woT_d.rearrange("(o p) c -> p o c", p=P))
            wv_sbs = [wvg[k // 3][:, k % 3, :] for k in range(KC)]

            # gate the ew stream the same way (dummy pool tiles; real ew
            # DMAs rotate into these slots, semaphore-ordered after fill)
            for i in range(EW_BUFS):
                g = ew_pool.tile([P, 2, N], f16, tag="ew", name=f"ewgate{i}")
                nc.gpsimd.dma_start(g[0:1, 0:1, 0:64], xg[1][0:1, 0, 0:64])
            # ew view: [sc, p, h, n]
            ew_r = ew_d.rearrange("h (c p) n -> c p h n", p=P)

            # ---- PE warmup ------------------------------------------------
            # Zero-matmul stream covering preamble -> first-group landing,
            # so the HAM clock-gate un-throttles into the QKV-m0 stream.
            warm_ps = psum_s.tile([P, N], f32, tag="ps_s")
            for i in range(NWARM):
                mm(warm_ps[:, 0:P], warm_sb, warm_sb,
                   start=(i == 0), stop=(i == NWARM - 1))

            qT_sbs = [const_pool.tile([P, N], f16, name=f"qT{j}")
                      for j in range(MQ)]
            kT_sbs = [const_pool.tile([P, N], f16, name=f"kT{j}")
                      for j in range(MQ)]
            oT_sbs = [const_pool.tile([P, N], f16, name=f"oT{j}")
                      for j in range(MQ)]
            # [v_h | 1 | 0...] (even heads use cols 0:65) /
            # [0... | 1 | 0 | v_h] (odd heads use cols 0:128, one at col 32)
            v_aug = const_pool.tile([P, SC, HPG, P], f16)
            for h in range(HPG):
                if h % 2 == 0:
                    nc.scalar.memzero(v_aug[:, :, h, 64:66])
                    nc.scalar.add(v_aug[:, :, h, 64:65],
                                  v_aug[:, :, h, 64:65], 1.0)
                else:
                    nc.scalar.memzero(v_aug[:, :, h, 0:64])
                    nc.scalar.add(v_aug[:, :, h, 32:33],
                                  v_aug[:, :, h, 32:33], 1.0)

            # ---- QKV projections ------------------------------------------
            # m0 streams behind the fill; casts split in halves so pair 0's
            # first steps unblock one ACT-copy earlier.
            def qkv_m0():
                ps_q = psum_s.tile([P, N], f32, tag="ps_s", name="qkv_q0")
                ps_k = psum_s.tile([P, N], f32, tag="ps_s", name="qkv_k0")
                for kc in range(KC):
                    # k first: pair 0's first qk LDWEIGHTS needs kT
                    for ps, j0 in ((ps_k, GJ), (ps_q, 0)):
                        for nb in range(NB2):
                            ncol = slice(nb * 512, (nb + 1) * 512)
                            mm(ps[:, ncol],
                               wqk_sbs[kc][:, j0:j0 + P],
                               x_sbs[kc][:, ncol],
                               start=(kc == 0), stop=(kc == KC - 1))
                # casts split in halves, k first (pair 0's first qk needs
                # kT's low half before qT's high half)
                for half in range(NB2):
                    ncol = slice(half * 512, (half + 1) * 512)
                    nc.scalar.copy(kT_sbs[0][:, ncol], ps_k[:, ncol])
                    nc.scalar.copy(qT_sbs[0][:, ncol], ps_q[:, ncol])

            qkv_m0()

            def qkv_m1(m, which):
                """Emit one of q/k for row-chunk m (1 psum slot borrow)."""
                j0 = m * P if which == "q" else GJ + m * P
                dst = qT_sbs[m] if which == "q" else kT_sbs[m]
                ps = psum_s.tile([P, N], f32, tag="ps_s",
                                 name=f"qkv_{which}{m}")
                for nb in range(NB2):
                    ncol = slice(nb * 512, (nb + 1) * 512)
                    for kc in range(KC):
                        mm(ps[:, ncol], wqk_sbs[kc][:, j0:j0 + P],
                           x_sbs[kc][:, ncol],
                           start=(kc == 0), stop=(kc == KC - 1))
                # split the two casts of each m-chunk across engines (a
                # full-tile DVE cast costs 1.7us, an ACT one 1.15us; both
                # engines are near their window budgets)
                if which == "q":
                    nc.scalar.copy(dst[:], ps)
                else:
                    nc.vector.tensor_copy(dst[:], ps)

            def emit_v(sc):
                # v-projection for one s-chunk on a ps_o slot; ScalarE
                # scatters the result into v_aug (even cols 0:64 / odd
                # cols 64:128 per head).
                ps = psum_o.tile([P, 512], f32, tag="ps_o", name=f"ps_v{sc}")
                for kc in range(KC):
                    mm(ps[:, :GJ],
                       x_sbs[kc][:, sc * P:(sc + 1) * P],
                       wv_sbs[kc][:, :],
                       start=(kc == 0), stop=(kc == KC - 1))
                vsrc = ps[:, :GJ].rearrange("p (h d) -> p h d", d=D)
                nc.vector.tensor_copy(v_aug[:, sc, 0:HPG:2, 0:64],
                                      vsrc[:, 0:HPG:2, :])
                nc.vector.tensor_copy(v_aug[:, sc, 1:HPG:2, 64:128],
                                      vsrc[:, 1:HPG:2, :])

            # ---- attention pair loop --------------------------------------
            def st_pair(hp, pv_sched=None, pv_emit=None, hooks_pre=None,
                        hooks=None, ets_out=None, prefetch_ew=False,
                        gp_mul_scs=()):
                """One even/odd head pair's qk -> exp -> (x ew) stream.
                hooks_pre[sc]: dense PE fills emitted BEFORE the qk mms
                (must not depend on late startup DMAs).  hooks[sc]: work
                emitted after the step's mul (may have late deps).
                pv_sched[sc] PV groups of pv_emit interleave after qk."""
                j = hp // 2
                qe, ke = qT_sbs[j][0:64, :], kT_sbs[j][0:64, :]
                qo, ko = qT_sbs[j][64:128, :], kT_sbs[j][64:128, :]
                ets_e, ets_o = ([], []) if ets_out is None else ets_out
                pv_next = 0
                ew_ts = []
                if prefetch_ew:
                    # split across two queues so the 4MB burst drains two
                    # rings in parallel; sync (whose FIFO also carries the
                    # pair-1 norm-chain hops) gets the late-needed tiles
                    for sc in range(SC):
                        ew_t = ew_pool.tile([P, 2, N], f16, tag="ew")
                        dq = nc.gpsimd if sc < 4 else nc.sync
                        dq.dma_start(ew_t, ew_r[sc][:, hp:hp + 2, :])
                        ew_ts.append(ew_t)
                for sc in range(SC):
                    scol = slice(sc * P, (sc + 1) * P)
                    if prefetch_ew:
                        ew_t = ew_ts[sc]
                    else:
                        ew_t = ew_pool.tile([P, 2, N], f16, tag="ew")
                        nc.gpsimd.dma_start(ew_t, ew_r[sc][:, hp:hp + 2, :])
                    if hooks_pre is not None and sc in hooks_pre:
                        for fn in hooks_pre[sc]:
                            fn()
                    ps_e = psum_s.tile([P, N], f32, tag="ps_s")
                    ps_o = psum_s.tile([P, N], f32, tag="ps_s")
                    # each 512-col psum region is written by exactly one
                    # matmul -> start+stop per region
                    for nb in range(NB2):
                        ncol = slice(nb * 512, (nb + 1) * 512)
                        mm(ps_e[:, ncol], ke[:, scol], qe[:, ncol],
                           start=True, stop=True)
                    for nb in range(NB2):
                        ncol = slice(nb * 512, (nb + 1) * 512)
                        mm(ps_o[:, ncol], ko[:, scol], qo[:, ncol],
                           start=True, stop=True)
                    if pv_emit is not None and pv_sched is not None:
                        for _ in range(pv_sched[sc]):
                            pv_emit(pv_next)
                            pv_next += 1
                    er_e = eraw_pool.tile([P, N], f16, tag="eraw")
                    nc.scalar.activation(er_e, ps_e, EXP)
                    er_o = eraw_pool.tile([P, N], f16, tag="eraw")
                    nc.scalar.activation(er_o, ps_o, EXP)
                    eng_o = nc.gpsimd if sc in gp_mul_scs else nc.vector
                    et_e = e_pool.tile([P, N], f16, tag="et")
                    nc.vector.tensor_mul(et_e, er_e, ew_t[:, 0, :])
                    et_o = e_pool.tile([P, N], f16, tag="et")
                    eng_o.tensor_mul(et_o, er_o, ew_t[:, 1, :])
                    ets_e.append(et_e)
                    ets_o.append(et_o)
                    if hooks is not None and sc in hooks:
                        for fn in hooks[sc]:
                            fn()
                return ets_e, ets_o, pv_next

            def make_pv(hp, ets_pair):
                halves = {}
                for h, _ in ets_pair:
                    for nb in range(NB2):
                        halves[(h, nb)] = psum_o.tile([P, 512], f32,
                                                      tag="ps_o",
                                                      name=f"pso_h{h}_n{nb}")

                def emit(sc):
                    for h, etiles in ets_pair:
                        even = (h % 2 == 0)
                        lh = (v_aug[:, sc, h, 0:65] if even
                              else v_aug[:, sc, h, 0:P])
                        for nb in range(NB2):
                            ncol = slice(nb * 512, (nb + 1) * 512)
                            pso = halves[(h, nb)]
                            po = (pso[0:65, :] if even else pso[:, :])
                            mm(po, lh, etiles[sc][:, ncol],
                               start=(sc == 0), stop=(sc == SC - 1))

                def emit_h(which, sc):
                    h, etiles = ets_pair[which]
                    even = (h % 2 == 0)
                    lh = (v_aug[:, sc, h, 0:65] if even
                          else v_aug[:, sc, h, 0:P])
                    for nb in range(NB2):
                        ncol = slice(nb * 512, (nb + 1) * 512)
                        pso = halves[(h, nb)]
                        po = (pso[0:65, :] if even else pso[:, :])
                        mm(po, lh, etiles[sc][:, ncol],
                           start=(sc == 0), stop=(sc == SC - 1))

                h_e, h_o = ets_pair[0][0], ets_pair[1][0]
                return emit, ([halves[(h_e, 0)], halves[(h_e, 1)]],
                              [halves[(h_o, 0)], halves[(h_o, 1)]]), emit_h

            # Norm chain (pairs 0/1), split into 3 stages so each DVE op's
            # inputs are resolved before it reaches the head of the
            # strict-FIFO DVE queue (an unresolved reciprocal would block
            # the exp-tile multiply stream for ~2us).  End-to-end latency
            # is ~11us (4 HBM round trips) — launched 1.5 windows before
            # the tail needs oT.
            def norm_a(h, halves, dq):
                # ONE copy per psum half evacuates the PV rows AND the r
                # row together (even head: psum rows 0:65 hold v|r; odd:
                # rows 32:128 hold r|..|v) — r-only copies would be
                # single-partition and ~0.7us each on DVE, which congests
                # the pair boundary.
                off = (h % 2) * 64
                rrow = 64 if h % 2 == 0 else 32
                vcp = vcp_pool.tile([P, N], f32, tag="vcp")
                for nb, pso in enumerate(halves):
                    ncol = slice(nb * 512, (nb + 1) * 512)
                    nc.vector.tensor_copy(vcp[:, ncol], pso)
                rd1 = dram_pool.tile([1, N], f32, tag="rd1")
                dq.dma_start(rd1, vcp[rrow:rrow + 1, :])
                rsq = r_pool.tile([P, N // P], f32, tag="rsq")
                dq.dma_start(
                    rsq, rd1.rearrange("one (p o) -> (one p) o", p=P))
                return rsq, vcp

            def norm_b(h, st, dq):
                rsq, vcp = st
                off = (h % 2) * 64
                nc.vector.reciprocal(rsq, rsq)
                rd2 = dram_pool.tile([1, N], f32, tag="rd2")
                dq.dma_start(
                    rd2.rearrange("one (p o) -> (one p) o", p=P), rsq)
                rb = rb_pool.tile([P, N], f32, tag="rb")
                dq.dma_start(rb[off:off + 64, :],
                             rd2[0:1, :].partition_broadcast(64))
                return rb, vcp

            def norm_c(h, st, eng=None):
                rb, vcp = st
                eng = eng or nc.vector
                off = (h % 2) * 64
                for nb in range(NB2):
                    ncol = slice(nb * 512, (nb + 1) * 512)
                    eng.tensor_mul(
                        oT_sbs[h // 2][off:off + 64, ncol],
                        vcp[off:off + 64, ncol],
                        rb[off:off + 64, ncol])

            # ---- the three pair windows -----------------------------------
            # v(0)/v(1) pre-emitted: they fill the PE-idle window between
            # the last m0 matmul and pair-0's first qk (blocked on the m0
            # casts); wv was gated on the first x group so it has landed.
            emit_v(0)
            emit_v(1)

            # pair 0 (heads 0/1): every step carries >=4k filler columns
            # (HAM re-warm needs fully-busy 3.4us windows): v chunks at
            # steps 0-3 and 6-7, the m1 groups at 4/5.  v(6)/v(7) are only
            # consumed by the PV0 burst late in pair 1, so they can land
            # last.
            hooks0_pre = {4: [lambda: qkv_m1(1, "q")],
                          5: [lambda: qkv_m1(1, "k")]}
            hooks0 = {0: [lambda: emit_v(2)], 1: [lambda: emit_v(3)],
                      2: [lambda: emit_v(4)], 3: [lambda: emit_v(5)],
                      6: [lambda: emit_v(6)], 7: [lambda: emit_v(7)]}
            ets0_e, ets0_o, _ = st_pair(0, hooks_pre=hooks0_pre,
                                        hooks=hooks0)

            # pair 1 (heads 2/3): PV(pair 0) spread (closes step 5), m2
            # fills at 5/6; pair-0 evacuations right at close (steps 5/6)
            # so the ps_o slots are free before pair-2's PV allocates;
            # the ~11us chain latency resolves mid-pair-2.
            st = {}
            pv0, (h0_e, h0_o), _ = make_pv(0, [(0, ets0_e), (1, ets0_o)])
            hooks1_pre = {
                5: [lambda: qkv_m1(2, "q")],
                # a(1) as a PRE-hook: its DVE copies run before step 6's
                # et-muls so the freed ps_o slots (and the mul stream) are
                # not held behind each other at the pair boundary.
                6: [lambda: st.__setitem__(1, norm_a(1, h0_o, nc.sync)),
                    lambda: qkv_m1(2, "k")],
            }
            hooks1 = {
                5: [lambda: st.__setitem__(0, norm_a(0, h0_e, nc.sync))],
                6: [lambda: st.__setitem__(0, norm_b(0, st[0], nc.sync))],
                7: [lambda: st.__setitem__(1, norm_b(1, st[1], nc.sync))],
            }
            ets1_e, ets1_o, _ = st_pair(
                2, pv_sched=[2, 2, 1, 1, 1, 1, 0, 0], pv_emit=pv0,
                hooks_pre=hooks1_pre, hooks=hooks1)

            # pair 2 (heads 4/5): all 8 ew tiles prefetched at step 0 on
            # two queues.  PV(1) burst closes at step 3; pair-1 norms
            # staged 4-7 (sync + gpsimd in parallel, landing ~tail+1);
            # pair-2's own PV self-lags from step 5 into the freed ps_o
            # slots.  Pair-0's final muls at steps 2/3 (their rb lands
            # around step 2 — emitted later so the strict-FIFO DVE queue
            # never parks on them).
            pv1, (h1_e, h1_o), pv1_h = make_pv(1, [(2, ets1_e), (3, ets1_o)])
            pv2_emit_holder = []

            def start_pv2():
                emit2, halves, _ = make_pv(2, [(4, ets2_e), (5, ets2_o)])
                pv2_emit_holder.append((emit2, halves))

            ets2_e, ets2_o = [], []

            def pv2(i):
                pv2_emit_holder[0][0](i)

            # DVE is the scarce engine in this window (et-muls + psum
            # evacuations + reciprocals ≈ its full budget): all six final
            # oT muls ride the idle GpSimd instead, and the pair-1 norm
            # chains ride gpsimd/sync where nothing queues behind them.
            # PV2 closes IN-LOOP at step 7 so the tail's 1/r chain starts
            # immediately; steps 5-7 are packed with its burst.
            hooks2_pre = {
                4: [lambda: st.__setitem__(2, norm_a(2, h1_e, nc.gpsimd))],
            }
            hooks2 = {
                2: [lambda: norm_c(0, st[0], nc.gpsimd)],
                3: [lambda: norm_c(1, st[1], nc.gpsimd)],
                4: [lambda: st.__setitem__(3, norm_a(3, h1_o, nc.sync))],
                5: [lambda: st.__setitem__(2, norm_b(2, st[2], nc.gpsimd)),
                    lambda: st.__setitem__(3, norm_b(3, st[3], nc.sync)),
                    start_pv2, lambda: pv2(0), lambda: pv2(1)],
                6: [lambda: norm_c(2, st[2], nc.gpsimd),
                    lambda: pv2(2), lambda: pv2(3), lambda: pv2(4)],
                7: [lambda: norm_c(3, st[3], nc.gpsimd),
                    lambda: pv2(5), lambda: pv2(6), lambda: pv2(7)],
            }
            for sc in range(4):
                hooks2.setdefault(0, []).append(
                    lambda sc=sc: pv1_h(0, sc))
                hooks2.setdefault(1, []).append(
                    lambda sc=sc: pv1_h(1, sc))
                hooks2.setdefault(2, []).append(
                    lambda sc=sc: pv1_h(0, sc + 4))
                hooks2.setdefault(3, []).append(
                    lambda sc=sc: pv1_h(1, sc + 4))
            st_pair(4, hooks_pre=hooks2_pre, hooks=hooks2,
                    ets_out=(ets2_e, ets2_o), prefetch_ew=True)
            _, (h2_e, h2_o) = pv2_emit_holder[0]

            # ---- tail -----------------------------------------------------

            def oproj_mms(nb, ps0, ps1, j3s, start, stop):
                for cb, ps in ((0, ps0), (1, ps1)):
                    cw = 512 if cb == 0 else C - 512
                    for j3 in j3s:
                        mm(ps[:, 0:cw],
                           oT_sbs[j3][:, nb * P:(nb + 1) * P],
                           woT_sb[:, j3, cb * 512:cb * 512 + cw],
                           start=(start and j3 == j3s[0]),
                           stop=(stop and j3 == j3s[-1]))

            # oproj j3=0,1 pre-run for nb 0/1 on the freed ps_s slots: PE
            # work that covers the final norm's ACT latency.
            pre = {}
            for nb in range(2):
                psw = psum_s.tile([P, N], f32, tag="ps_s", name=f"pow_{nb}")
                pre[nb] = (psw[:, 0:512], psw[:, 512:1024])
                oproj_mms(nb, pre[nb][0], pre[nb][1], [0, 1], True, False)
            # nb2 pre-runs on TWO ps_o slots (freed by the evacuations
            # below); the 1/r broadcasts use the other two, per-head
            # sequentially — no slot set is ever held against oT[2].
            po2 = (psum_o.tile([P, 512], f32, tag="ps_o", name="po0_2"),
                   psum_o.tile([P, 512], f32, tag="ps_o", name="po1_2"))
            pre[2] = po2
            oproj_mms(2, po2[0], po2[1], [0, 1], True, False)

            # Final pair's norm, DMA-free: 1/r = exp(-ln r) on ACT.  Both
            # heads' r rows (psum partitions 64 / 32) are copied into ONE
            # tile so a SINGLE Ln and a SINGLE Exp over partition rows
            # 32:65 cover both heads — one table-set switch each instead
            # of the 4 the scheduler otherwise interleaves (rows 33:63
            # process garbage; their outputs are never read).  A K=1
            # ones-matmul then broadcasts each head's 1/r row across its
            # 64 output partitions via PSUM.
            # ALL r-row copies on ACT: a DVE r-copy at the strict-FIFO head
            # would block the final et-multiplies (which gate PV2's close);
            # ACT is otherwise idle here and the Ln follows on the same
            # queue.  Odd head (row 32) first — its PV closed first.
            r_t = r_pool.tile([P, N], f32, tag="r", name="rt_tail")
            for h, halves in ((5, h2_o), (4, h2_e)):
                rrow = 64 if h % 2 == 0 else 32
                for nb, pso in enumerate(halves):
                    ncol = slice(nb * 512, (nb + 1) * 512)
                    nc.scalar.copy(r_t[rrow:rrow + 1, ncol],
                                   pso[rrow:rrow + 1, :])
            vcps = {}
            for h, halves in ((4, h2_e), (5, h2_o)):
                off = (h % 2) * 64
                vcp = vcp_pool.tile([P, N], f32, tag="vcp", name=f"vcpt{h}")
                for nb, pso in enumerate(halves):
                    ncol = slice(nb * 512, (nb + 1) * 512)
                    nc.vector.tensor_copy(vcp[off:off + 64, ncol],
                                          pso[off:off + 64, :])
                vcps[h] = vcp
            # full-height Ln/Exp: ACT time is free-dim-paced, so covering
            # all 128 partitions costs the same as 2 rows and keeps the
            # AP legal; rows other than 32/64 process garbage, unread.
            rln_t = r_pool.tile([P, N], f32, tag="rsq2", name="rln_tail")
            nc.scalar.activation(rln_t, r_t, LOG)
            rinv_t = r_pool.tile([P, N], f16, tag="rfl", name="rinv_tail")
            nc.scalar.activation(rinv_t, rln_t, EXP, scale=-1.0)
            # 1/r broadcast via K=1 ones-matmuls into the ps_o slots the
            # evacuations just freed, then the oT[2] muls on DVE.
            for h in (4, 5):
                off = (h % 2) * 64
                rrow = 64 if h % 2 == 0 else 32
                for nb in range(NB2):
                    ncol = slice(nb * 512, (nb + 1) * 512)
                    rbp = psum_o.tile([P, 512], f32, tag="ps_o",
                                      name=f"rbp{h}_{nb}")
                    mm(rbp[off:off + 64, :], ones_sb[rrow:rrow + 1, 0:64],
                       rinv_t[rrow:rrow + 1, ncol], start=True, stop=True)
                    nc.vector.tensor_mul(
                        oT_sbs[h // 2][off:off + 64, ncol],
                        vcps[h][off:off + 64, ncol],
                        rbp[off:off + 64, :])

            def oproj_evac(nb, ps0, ps1):
                # evacuations pace the tail if they all ride one engine:
                # split halves across DVE and ACT (the DVE psum->fp16 cast
                # is contiguous here, which the hardware handles; only
                # strided casts mis-stride).
                ob = out_pool.tile([P, C], f16, tag="ob")
                nc.vector.tensor_copy(ob[:, 0:512], ps0)
                nc.scalar.copy(ob[:, 512:C], ps1[:, 0:C - 512])
                nc.sync.dma_start(
                    out_d.rearrange("(o p) c -> o p c", p=P)[nb], ob)

            for nb in range(SC):
                if nb in pre:
                    ps0, ps1 = pre[nb]
                    oproj_mms(nb, ps0, ps1, [2], False, True)
                else:
                    ps0 = psum_o.tile([P, 512], f32, tag="ps_o",
                                      name=f"po0_{nb}")
                    ps1 = psum_o.tile([P, 512], f32, tag="ps_o",
                                      name=f"po1_{nb}")
                    oproj_mms(nb, ps0, ps1, [0, 1, 2], True, True)
                oproj_evac(nb, ps0, ps1)

    nc.compile()
    return nc


_PROG = None


def _get_prog():
    global _PROG
    if _PROG is None:
        _PROG = build_program()
    return _PROG


def make_in_maps(query, attn_weight, Wq, Wk, Wv, Wo):
    query = np.asarray(query, dtype=np.float32)
    attn_weight = np.asarray(attn_weight, dtype=np.float32)
    Wq = np.asarray(Wq, dtype=np.float32)
    Wk = np.asarray(Wk, dtype=np.float32)
    Wv = np.asarray(Wv, dtype=np.float32)
    Wo = np.asarray(Wo, dtype=np.float32)

    in_maps = []
    for b in range(B):
        xT = np.ascontiguousarray(query[b].T).astype(np.float16)
        for g in range(HG):
            rows = slice(g * GJ, (g + 1) * GJ)
            wqk = np.ascontiguousarray(np.concatenate(
                [(SCALE * Wq[rows, :]).T, Wk[rows, :].T],
                axis=1)).astype(np.float16)
            wvT = np.ascontiguousarray(Wv[rows, :].T).astype(np.float16)
            woT = np.ascontiguousarray(Wo[:, rows].T).astype(np.float16)
            ew = np.exp(np.ascontiguousarray(
                attn_weight[b, g * HPG:(g + 1) * HPG].transpose(0, 2, 1))
            ).astype(np.float16)
            in_maps.append({
                "xT": xT, "wqk": wqk, "wvT": wvT, "woT": woT, "ew": ew,
            })
    return in_maps


def run(inputs, trace=False, **spmd_kwargs):
    """Execute on 8 cores; returns (full_output, BassKernelResults)."""
    from concourse import bass_utils

    nc = _get_prog()
    in_maps = make_in_maps(inputs["query"], inputs["attn_weight"],
                           inputs["Wq"], inputs["Wk"], inputs["Wv"],
                           inputs["Wo"])
    res = bass_utils.run_bass_kernel_spmd(
        nc, in_maps, core_ids=list(range(NCORES)), trace=trace, **spmd_kwargs)
    bo = np.asarray(inputs["bo"], dtype=np.float32)
    full = np.empty((B, N, C), dtype=np.float32)
    for b in range(B):
        full[b] = (res.results[2 * b]["out"].astype(np.float32)
                   + res.results[2 * b + 1]["out"].astype(np.float32) + bo)
    return full, res


def kernel(**inputs):
    full, _ = run(inputs, trace=False)
    return full
